# revision 1
# baseline (speedup 1.0000x reference)
"""Trainium2 Bass kernel for supervised contrastive loss (8-core SPMD).

Math (per reference):
    f = x / max(||x||, 1e-12)            row-normalized features  [B, D]
    s = (f f^T) / TEMP                                            [B, B]
    E = exp(s) with diag zeroed
    P_i = sum_{j != i, l_j == l_i} E_ij   (positives)
    T_i = sum_{j != i} E_ij               (positives + negatives)
    loss = mean_i [ log(T_i + EPS) - log(P_i) ]

Distribution: row-block shard. Core c owns rows m in [1024c, 1024(c+1)).
Each core computes E^T blocks [j-chunk(128) x m(1024)] with j on the
partition dim, so BOTH reductions (over j) are partition-contractions and
run on the TensorEngine:
    PS1[c', m] = sum_j Y'[j, c'] * E[j, m]     (Y' = one-hot(labels) | ones)
row 100 of PS1 = T_m, and P_m = PS1[l_m, m] (recovered with a one-hot
mask + ones-matmul). Per-core scalar partial losses are summed on host.

SPMD uniformity: every core runs the identical program; per-core variation
lives entirely in the input data. Chunk arrays are rotated per core so the
core's own (diagonal) chunks are always processed at t = 0..7.
"""

import numpy as np
import ml_dtypes

TEMPERATURE = 0.07
EPS = 1e-8
B = 8192
D = 512
NCORES = 8
M = B // NCORES          # 1024 rows per core
NCH = B // 128           # 64 j-chunks of 128
BCH = M // 128           # 8 chunks belonging to the core's own block
NCLS = 100               # label classes
YC = NCLS + 1            # one-hot columns + ones column

_CACHE = {}


def _build_bass():
    import concourse.bass as bass
    import concourse.bacc as bacc
    import concourse.tile as tile
    from concourse import mybir
    from contextlib import ExitStack

    f32 = mybir.dt.float32
    bf16 = mybir.dt.bfloat16
    AF = mybir.ActivationFunctionType
    OP = mybir.AluOpType

    nc = bacc.Bacc()

    # ---- I/O ----------------------------------------------------------
    # xt[t, p, dc*128+jj] = x[jc_t*128+jj, dc*128+p]   (chunk-major x^T)
    xt_d = nc.declare_dram_parameter("xt", [NCH, 128, D], bf16, isOutput=False)
    # xnat[t, p, d] = x[jc_t*128+p, d]                 (natural row tiles)
    xnat_d = nc.declare_dram_parameter("xnat", [NCH, 128, D], bf16, isOutput=False)
    # all label/iota constants in ONE tensor (single DMA -> single wait for
    # downstream DVE ops, which only support one sync-wait in walrus):
    #   [:, 0:6464]        iota[p, t, c'] = c' - 1
    #   [:, 6464:12928]    labbc[p, t, c'] = labels[jc_t*128+p]
    #   [:, 12928:13952]   labblk[p, m] = labels[block row m]
    #   [:, 13952:13954]   iotap[p] = p - 1 as raw f32 (two bf16 slots)
    LC = NCH * YC
    labio_d = nc.declare_dram_parameter(
        "labio", [128, 2 * LC + M + 2], bf16, isOutput=False
    )
    loss_d = nc.declare_dram_parameter("loss", [1, 1], f32, isOutput=True)

    with ExitStack() as ctx:
        tc = ctx.enter_context(tile.TileContext(nc))
        const = ctx.enter_context(tc.tile_pool(name="const", bufs=1))
        xtp = ctx.enter_context(tc.tile_pool(name="xtp", bufs=4))
        xnp = ctx.enter_context(tc.tile_pool(name="xnp", bufs=4))
        sqp = ctx.enter_context(tc.tile_pool(name="sqp", bufs=4))
        lnp = ctx.enter_context(tc.tile_pool(name="lnp", bufs=2))
        ep = ctx.enter_context(tc.tile_pool(name="ep", bufs=3))
        psum = ctx.enter_context(tc.tile_pool(name="psum", bufs=3, space="PSUM"))
        accp = ctx.enter_context(tc.tile_pool(name="accp", bufs=1, space="PSUM"))

        # ---- constants / label machinery ------------------------------
        labio = const.tile([128, 2 * LC + M + 2], bf16)
        nc.sync.dma_start(out=labio[:], in_=labio_d[:])
        iota_cl = labio[:, 0:LC].rearrange("p (t c) -> p t c", c=YC)
        labbc_sb = labio[:, LC : 2 * LC].rearrange("p (t c) -> p t c", c=YC)
        labblk_sb = labio[:, 2 * LC : 2 * LC + M]
        iota_p = labio[:, 2 * LC + M : 2 * LC + M + 2].bitcast(f32)

        # Y'[p, t, c'] = (c'-1 == labels[j]) for c' in 1..100; col 0 = ones
        # (T-sum column at c'=0 so T lands on PSUM partition 0).
        yall = const.tile([128, NCH, YC], bf16)
        nc.vector.tensor_tensor(
            out=yall[:], in0=iota_cl, in1=labbc_sb, op=OP.is_equal
        )
        nc.vector.memset(yall[:, :, 0:1], 1.0)

        # YblkT[c', m] = (labels[block m] == c'-1)
        yblkt = const.tile([128, M], bf16)
        nc.vector.tensor_scalar(
            out=yblkt[:], in0=labblk_sb, scalar1=iota_p, scalar2=None,
            op0=OP.is_equal,
        )

        ones101 = const.tile([128, 1], f32)
        nc.vector.memset(ones101[:], 1.0)
        bias_ltemp = const.tile([128, 1], f32)
        nc.vector.memset(bias_ltemp[:], float(-np.log(TEMPERATURE)))
        bias_eps = const.tile([128, 1], f32)
        nc.vector.memset(bias_eps[:], EPS)

        # ---- row norms:  nsq[j] = sum_d x[j,d]^2  (grouped by 8 chunks) ---
        nsqg = [
            const.tile([128, BCH], f32, tag=f"nsq{g}", name=f"nsq{g}")
            for g in range(8)
        ]
        scaleg = [
            const.tile([128, BCH], f32, tag=f"scl{g}", name=f"scl{g}")
            for g in range(8)
        ]
        for t in range(NCH):
            xn_t = xnp.tile([128, D], bf16)
            nc.sync.dma_start(out=xn_t[:], in_=xnat_d[t])
            sq_t = sqp.tile([128, D], bf16)
            # square + free-dim reduce on DVE, keeping ACT free for the exps
            nc.vector.tensor_mul(out=sq_t[:], in0=xn_t[:], in1=xn_t[:])
            nc.vector.tensor_reduce(
                out=nsqg[t // BCH][:, t % BCH : t % BCH + 1], in_=sq_t[:],
                axis=mybir.AxisListType.X, op=OP.add,
            )

        # scale_j = 1/(||x_j|| * TEMP) = exp(-0.5*ln(nsq) - ln(TEMP))
        for g in range(8):
            ln_g = lnp.tile([128, BCH], f32)
            nc.scalar.activation(out=ln_g[:], in_=nsqg[g][:], func=AF.Ln)
            nc.scalar.activation(
                out=scaleg[g][:], in_=ln_g[:], func=AF.Exp,
                bias=bias_ltemp[:], scale=-0.5,
            )

        # ---- normalized own-block x^T:  xnT[d, m] = x^T[d, m] / ||x_m|| ---
        x8 = const.tile([128, BCH, D], bf16)
        nc.sync.dma_start(
            out=x8[:], in_=xt_d[0:BCH].rearrange("t p f -> p t f")
        )

        # block-row norms in ROW layout: nsq_row[0, m] = sum_{p,dc} xT[.,m]^2
        # via DVE square + 4 accumulated ones-matmul partition reductions.
        ones_bf = const.tile([128, 1], bf16)
        nc.vector.memset(ones_bf[:], 1.0)
        x8sq = const.tile([128, BCH, D], bf16)
        nc.vector.tensor_tensor(
            out=x8sq[:], in0=x8[:], in1=x8[:], op=OP.mult
        )
        nsqrow_ps = psum.tile([128, M], f32, tag="sim", name="nsqrow_ps")
        for dc in range(4):
            for h in range(2):
                nc.tensor.matmul(
                    nsqrow_ps[0:1, h * 512 : (h + 1) * 512],
                    lhsT=ones_bf[:, 0:1],
                    rhs=x8sq[:, h * 4 : (h + 1) * 4, dc * 128 : (dc + 1) * 128],
                    start=(dc == 0),
                    stop=(dc == 3),
                )
        lnrow = const.tile([1, M], f32)
        nc.scalar.activation(out=lnrow[:], in_=nsqrow_ps[0:1, :], func=AF.Ln)
        # row of 1/||x_m|| on partition 0 of a zeroed tile, then broadcast to
        # all partitions with a ones-matmul (K=128, rows 1..127 are zero).
        rowpad = const.tile([128, M], f32)
        nc.vector.memset(rowpad[:], 0.0)
        nc.scalar.activation(
            out=rowpad[0:1, :], in_=lnrow[:], func=AF.Exp, bias=0.0, scale=-0.5
        )
        ones_f = const.tile([128, 128], f32)
        nc.vector.memset(ones_f[:], 1.0)
        invnbc_ps = psum.tile([128, M], f32, tag="sim", name="invnbc_ps")
        for h in range(2):
            nc.tensor.matmul(
                invnbc_ps[:, h * 512 : (h + 1) * 512],
                lhsT=ones_f[:],
                rhs=rowpad[:, h * 512 : (h + 1) * 512],
                start=True,
                stop=True,
            )
        # DVE copy PSUM->SBUF so the xnt multiplies have a single
        # cross-engine dependency (the x8 DMA).
        invnbc = const.tile([128, M], f32)
        nc.vector.tensor_copy(out=invnbc[:], in_=invnbc_ps[:])
        xnt = const.tile([128, 4, M], bf16)
        for dc in range(4):
            nc.vector.tensor_tensor(
                out=xnt[:, dc, :].rearrange("p (t j) -> p t j", j=128),
                in0=x8[:, :, dc * 128 : (dc + 1) * 128],
                in1=invnbc[:].rearrange("p (t j) -> p t j", j=128),
                op=OP.mult,
            )

        # ---- main loop over j-chunks ----------------------------------
        ps1 = accp.tile([128, M], f32)  # row 0: T; rows 1..100: class sums
        for t in range(NCH):
            if t < BCH:
                lhs = x8[:, t, :]
            else:
                lhs_t = xtp.tile([128, D], bf16)
                nc.sync.dma_start(out=lhs_t[:], in_=xt_d[t])
                lhs = lhs_t[:]
            ps = psum.tile([128, M], f32, tag="sim")
            for dc in range(4):
                for h in range(2):
                    nc.tensor.matmul(
                        ps[:, h * 512 : (h + 1) * 512],
                        lhsT=lhs[:, dc * 128 : (dc + 1) * 128],
                        rhs=xnt[:, dc, h * 512 : (h + 1) * 512],
                        start=(dc == 0),
                        stop=(dc == 3),
                    )
            e_t = ep.tile([128, M], bf16)
            g, k = t // BCH, t % BCH
            nc.scalar.activation(
                out=e_t[:], in_=ps[:], func=AF.Exp, scale=scaleg[g][:, k : k + 1]
            )
            if t < BCH:
                # zero the diagonal: kill (p, m) where m - p - 128*t == 0
                nc.gpsimd.affine_select(
                    out=e_t[:], in_=e_t[:], pattern=[[1, M]],
                    compare_op=OP.not_equal, fill=0.0,
                    base=-(t * 128), channel_multiplier=-1,
                )
            for h in range(2):
                nc.tensor.matmul(
                    ps1[0:YC, h * 512 : (h + 1) * 512],
                    lhsT=yall[:, t, :],
                    rhs=e_t[:, h * 512 : (h + 1) * 512],
                    start=(t == 0),
                    stop=(t == NCH - 1),
                )

        # ---- finalize: P via one-hot mask + partition reduce ----------
        maskd = const.tile([128, M], f32)
        nc.vector.tensor_tensor(
            out=maskd[0:YC, :], in0=ps1[0:YC, :], in1=yblkt[0:YC, :], op=OP.mult
        )
        pps = psum.tile([128, M], f32, tag="sim")
        for h in range(2):
            nc.tensor.matmul(
                pps[0:1, h * 512 : (h + 1) * 512],
                lhsT=ones101[0:YC, 0:1],
                rhs=maskd[0:YC, h * 512 : (h + 1) * 512],
                start=True,
                stop=True,
            )
        ln_t = const.tile([1, M], f32)
        nc.scalar.activation(
            out=ln_t[:], in_=ps1[0:1, :], func=AF.Ln, bias=bias_eps[0:1, :]
        )
        ln_p = const.tile([1, M], f32)
        nc.scalar.activation(out=ln_p[:], in_=pps[0:1, :], func=AF.Ln)
        diff = const.tile([1, M], f32)
        nc.vector.tensor_sub(out=diff[:], in0=ln_t[:], in1=ln_p[:])
        losss = const.tile([1, 1], f32)
        nc.vector.tensor_reduce(
            out=losss[:], in_=diff[:], axis=mybir.AxisListType.X, op=OP.add
        )
        nc.sync.dma_start(out=loss_d[:], in_=losss[:])

    # Bacc.finalize() runs the wait-splitting / ldweights / act-table /
    # extended-ISA codegen passes that walrus requires.
    nc.finalize()
    return nc


def _prep_inputs(features: np.ndarray, labels: np.ndarray):
    """Shard + lay out the full inputs for the 8 cores (host marshalling)."""
    bf16 = ml_dtypes.bfloat16
    x = np.ascontiguousarray(features, dtype=np.float32)
    x_bf = x.astype(bf16)
    # chunk-major x^T: xtc[jc, p, dc*128+jj] = x[jc*128+jj, dc*128+p]
    xtc = np.ascontiguousarray(
        x_bf.reshape(NCH, 128, 4, 128).transpose(0, 3, 2, 1)
    ).reshape(NCH, 128, D)
    xnat = x_bf.reshape(NCH, 128, D)
    lab_f = labels.astype(np.float32)
    lab_ch = lab_f.reshape(NCH, 128)
    LC = NCH * YC
    iota_cl = np.broadcast_to(
        (np.arange(YC, dtype=np.float32) - 1.0)[None, None, :], (128, NCH, YC)
    )
    iota_p = (np.arange(128, dtype=np.float32) - 1.0)[:, None]
    in_maps = []
    for c in range(NCORES):
        r = np.roll(np.arange(NCH), -BCH * c)
        labio = np.empty((128, 2 * LC + M + 2), dtype=np.float32)
        labio[:, 0:LC] = iota_cl.reshape(128, LC)
        # labbc[p, t, c'] = labels[jc_t*128 + p]
        labio[:, LC : 2 * LC] = np.repeat(lab_ch[r].T, YC, axis=1)
        labio[:, 2 * LC : 2 * LC + M] = lab_f[c * M : (c + 1) * M][None, :]
        labio_bf = labio.astype(bf16)
        # last two bf16 slots per row hold the raw f32 bits of (p - 1)
        labio_bf.view(np.uint16)[:, 2 * LC + M :] = (
            iota_p.astype("<f4").view(np.uint16).reshape(128, 2)
        )
        in_maps.append(
            {
                "xt": np.ascontiguousarray(xtc[r]),
                "xnat": np.ascontiguousarray(xnat[r]),
                "labio": labio_bf,
            }
        )
    return in_maps


def kernel(features: np.ndarray, labels: np.ndarray) -> np.ndarray:
    from concourse.bass_utils import run_bass_kernel_spmd

    if "nc" not in _CACHE:
        _CACHE["nc"] = _build_bass()
    nc = _CACHE["nc"]
    in_maps = _prep_inputs(features, labels)
    res = run_bass_kernel_spmd(nc, in_maps, list(range(NCORES)))
    total = sum(float(r["loss"][0, 0]) for r in res.results)
    return np.float32(total / B)



# revision 3
# speedup vs baseline: 13.0030x; 13.0030x over previous
"""Trainium2 Bass kernel for supervised contrastive loss (8-core SPMD).

Math (per reference):
    f = x / max(||x||, 1e-12)            row-normalized features  [B, D]
    s = (f f^T) / TEMP                                            [B, B]
    E = exp(s) with diag zeroed
    P_i = sum_{j != i, l_j == l_i} E_ij   (positives)
    T_i = sum_{j != i} E_ij               (positives + negatives)
    loss = mean_i [ log(T_i + EPS) - log(P_i) ]

Distribution: row-block shard with an on-device AllGather. The host
pre-normalizes (and folds in 1/sqrt(TEMP)) so each core is shipped ONLY its
own 1 MB bf16 shard in chunk-major transposed layout; the full [B, D]
operand is assembled on-device over NeuronLink. This cuts host->device
traffic ~18x vs replicating two full layouts per core, which dominated
end-to-end time under the axon tunnel.

Core c owns rows m in [1024c, 1024(c+1)). For each j-chunk (128 rows) it
computes the E^T block [j x m] with j on the partition dim so both masked
reductions contract over j on the TensorEngine:
    PS1[c', m] = sum_j Y'[j, c'] * E[j, m]     (Y' = one-hot(labels) | ones)
row 0 of PS1 = T_m, and P_m = PS1[l_m+1, m] (recovered with a one-hot
mask + ones-matmul). The diagonal is zeroed with a data-driven mask
(m == t*128 + p - 1024c), so every core runs the identical program with
per-core variation living only in the input data. Per-core scalar partial
losses are summed on host.

The jitted executable is cached in _CACHE: repeat kernel() calls pay only
input marshalling + transfer + device execution.
"""

import numpy as np
import ml_dtypes

TEMPERATURE = 0.07
EPS = 1e-8
B = 8192
D = 512
NCORES = 8
M = B // NCORES          # 1024 rows per core
NCH = B // 128           # 64 j-chunks of 128
BCH = M // 128           # 8 chunks per core shard
NCLS = 100               # label classes
YC = NCLS + 1            # one-hot columns + ones column
TW = NCH + YC + 1 + NCH  # tab cols: jadj | iota_c | iota_p | lab_ch

_CACHE = {}


def _build_bass():
    import concourse.bacc as bacc
    import concourse.tile as tile
    from concourse import mybir
    from contextlib import ExitStack

    f32 = mybir.dt.float32
    bf16 = mybir.dt.bfloat16
    AF = mybir.ActivationFunctionType
    OP = mybir.AluOpType

    nc = bacc.Bacc(num_devices=NCORES)

    # ---- I/O ----------------------------------------------------------
    # xs[t8, p, dc*128+jj] = xhat[(8c+t8)*128+jj, dc*128+p]  (shard, chunk-
    # major x^T; xhat = f / (max(||f||,1e-12) * sqrt(TEMP)) built on host)
    xs_d = nc.declare_dram_parameter("xs", [BCH, 128, D], bf16, isOutput=False)
    # tab[:, 0:64]      jadj[p, t] = t*128 + p - c*M
    # tab[:, 64:165]    iota_c[p, i] = i - 1
    # tab[:, 165:166]   iota_p[p] = p - 1
    # tab[:, 166:230]   lab_ch[p, t] = labels[t*128 + p]
    tab_d = nc.declare_dram_parameter("tab", [128, TW], f32, isOutput=False)
    # rows[0, m] = m ; rows[1, m] = labels[c*M + m]
    rows_d = nc.declare_dram_parameter("rows", [2, M], f32, isOutput=False)
    loss_d = nc.declare_dram_parameter("loss", [1, 1], f32, isOutput=True)

    with ExitStack() as ctx:
        tc = ctx.enter_context(tile.TileContext(nc))
        const = ctx.enter_context(tc.tile_pool(name="const", bufs=1))
        ep = ctx.enter_context(tc.tile_pool(name="ep", bufs=3))
        emp = ctx.enter_context(tc.tile_pool(name="emp", bufs=3))
        mkp = ctx.enter_context(tc.tile_pool(name="mkp", bufs=3))
        psum = ctx.enter_context(tc.tile_pool(name="psum", bufs=3, space="PSUM"))
        accp = ctx.enter_context(tc.tile_pool(name="accp", bufs=1, space="PSUM"))
        dram = ctx.enter_context(tc.tile_pool(name="dram", bufs=1, space="DRAM"))

        # ---- all-gather the shard into the full chunk-major x^T -------
        in_b = dram.tile([BCH, 128, D], bf16, name="in_b")
        out_b = dram.tile([NCH, 128, D], bf16, name="out_b", addr_space="Shared")
        nc.gpsimd.dma_start(out=in_b[:], in_=xs_d[:])
        nc.gpsimd.collective_compute(
            "AllGather",
            OP.bypass,
            replica_groups=[list(range(NCORES))],
            ins=[in_b[:].opt()],
            outs=[out_b[:].opt()],
        )
        # xall[p, t, f] = gathered[t, p, f]: 64 KB/partition, lives in SBUF
        xall = const.tile([128, NCH, D], bf16)
        nc.gpsimd.dma_start(out=xall[:], in_=out_b[:].rearrange("t p f -> p t f"))

        # own-block rhs: xnt[p, dc, t8*128+jj] = xs[t8, p, dc*128+jj]
        xnt4 = const.tile([128, 4, BCH, 128], bf16)
        nc.sync.dma_start(
            out=xnt4[:], in_=xs_d[:].rearrange("t p (dc j) -> p dc t j", j=128)
        )
        xnt = xnt4[:].rearrange("p dc t j -> p dc (t j)")

        # ---- constants / label machinery ------------------------------
        tab_s = const.tile([128, TW], f32)
        nc.sync.dma_start(out=tab_s[:], in_=tab_d[:])
        jadj = tab_s[:, 0:NCH]
        iota_c = tab_s[:, NCH : NCH + YC]
        iota_p = tab_s[:, NCH + YC : NCH + YC + 1]
        lab_ch = tab_s[:, NCH + YC + 1 : TW]

        # [1, M] rows land on partition 0 of zeroed pads, then are
        # broadcast to all partitions with a ones-matmul (K=128).
        rowpadA = const.tile([128, M], f32)
        nc.vector.memset(rowpadA[:], 0.0)
        nc.sync.dma_start(out=rowpadA[0:1, :], in_=rows_d[0:1, :])
        rowpadB = const.tile([128, M], f32)
        nc.vector.memset(rowpadB[:], 0.0)
        nc.sync.dma_start(out=rowpadB[0:1, :], in_=rows_d[1:2, :])

        ones_f = const.tile([128, 128], f32)
        nc.vector.memset(ones_f[:], 1.0)
        ones101 = const.tile([128, 1], f32)
        nc.vector.memset(ones101[:], 1.0)
        bias_eps = const.tile([128, 1], f32)
        nc.vector.memset(bias_eps[:], EPS)

        miota_ps = psum.tile([128, M], f32, tag="sim", name="miota_ps")
        for h in range(2):
            nc.tensor.matmul(
                miota_ps[:, h * 512 : (h + 1) * 512],
                lhsT=ones_f[:],
                rhs=rowpadA[:, h * 512 : (h + 1) * 512],
                start=True,
                stop=True,
            )
        miota_bc = const.tile([128, M], f32)
        nc.vector.tensor_copy(out=miota_bc[:], in_=miota_ps[:])

        labblk_ps = psum.tile([128, M], f32, tag="sim", name="labblk_ps")
        for h in range(2):
            nc.tensor.matmul(
                labblk_ps[:, h * 512 : (h + 1) * 512],
                lhsT=ones_f[:],
                rhs=rowpadB[:, h * 512 : (h + 1) * 512],
                start=True,
                stop=True,
            )
        labblk_bc = const.tile([128, M], f32)
        nc.vector.tensor_copy(out=labblk_bc[:], in_=labblk_ps[:])

        # Y'[p, t, c'] = (c'-1 == labels[t*128+p]) for c' in 1..100;
        # col 0 = ones (T-sum column, lands on PSUM partition 0).
        yall = const.tile([128, NCH, YC], bf16)
        for t in range(NCH):
            nc.vector.tensor_scalar(
                out=yall[:, t, :], in0=iota_c, scalar1=lab_ch[:, t : t + 1],
                scalar2=None, op0=OP.is_equal,
            )
        nc.vector.memset(yall[:, :, 0:1], 1.0)

        # YblkT[c', m] = (labels[c*M + m] == c'-1)
        yblkt = const.tile([128, M], bf16)
        nc.vector.tensor_scalar(
            out=yblkt[:], in0=labblk_bc[:], scalar1=iota_p,
            scalar2=None, op0=OP.is_equal,
        )

        # ---- main loop over j-chunks ----------------------------------
        ps1 = accp.tile([128, M], f32)  # row 0: T; rows 1..100: class sums
        for t in range(NCH):
            ps = psum.tile([128, M], f32, tag="sim")
            for dc in range(4):
                for h in range(2):
                    nc.tensor.matmul(
                        ps[:, h * 512 : (h + 1) * 512],
                        lhsT=xall[:, t, dc * 128 : (dc + 1) * 128],
                        rhs=xnt[:, dc, h * 512 : (h + 1) * 512],
                        start=(dc == 0),
                        stop=(dc == 3),
                    )
            e_t = ep.tile([128, M], bf16)
            nc.scalar.activation(out=e_t[:], in_=ps[:], func=AF.Exp)
            # diag mask: zero (p, m) where m == t*128 + p - c*M
            mask_t = mkp.tile([128, M], bf16)
            nc.vector.tensor_scalar(
                out=mask_t[:], in0=miota_bc[:], scalar1=jadj[:, t : t + 1],
                scalar2=None, op0=OP.not_equal,
            )
            em_t = emp.tile([128, M], bf16)
            nc.vector.tensor_mul(out=em_t[:], in0=e_t[:], in1=mask_t[:])
            for h in range(2):
                nc.tensor.matmul(
                    ps1[0:YC, h * 512 : (h + 1) * 512],
                    lhsT=yall[:, t, :],
                    rhs=em_t[:, h * 512 : (h + 1) * 512],
                    start=(t == 0),
                    stop=(t == NCH - 1),
                )

        # ---- finalize: P via one-hot mask + partition reduce ----------
        maskd = const.tile([128, M], f32)
        nc.vector.tensor_tensor(
            out=maskd[0:YC, :], in0=ps1[0:YC, :], in1=yblkt[0:YC, :], op=OP.mult
        )
        pps = psum.tile([128, M], f32, tag="sim", name="pps")
        for h in range(2):
            nc.tensor.matmul(
                pps[0:1, h * 512 : (h + 1) * 512],
                lhsT=ones101[0:YC, 0:1],
                rhs=maskd[0:YC, h * 512 : (h + 1) * 512],
                start=True,
                stop=True,
            )
        ln_t = const.tile([1, M], f32)
        nc.scalar.activation(
            out=ln_t[:], in_=ps1[0:1, :], func=AF.Ln, bias=bias_eps[0:1, :]
        )
        ln_p = const.tile([1, M], f32)
        nc.scalar.activation(out=ln_p[:], in_=pps[0:1, :], func=AF.Ln)
        diff = const.tile([1, M], f32)
        nc.vector.tensor_sub(out=diff[:], in0=ln_t[:], in1=ln_p[:])
        losss = const.tile([1, 1], f32)
        nc.vector.tensor_reduce(
            out=losss[:], in_=diff[:], axis=mybir.AxisListType.X, op=OP.add
        )
        nc.sync.dma_start(out=loss_d[:], in_=losss[:])

    # Bacc.finalize() runs the wait-splitting / ldweights / act-table /
    # extended-ISA codegen passes that walrus requires.
    nc.finalize()
    return nc


def _get_runner():
    """Build the Bass program and a CACHED jitted SPMD executable.

    run_bass_kernel_spmd builds a fresh jit closure per call (full retrace +
    XLA compile every time); caching the executable makes repeat kernel()
    calls pay only marshal + transfer + execute.
    """
    if "runner" in _CACHE:
        return _CACHE["runner"]
    import jax
    from jax.sharding import Mesh, PartitionSpec
    from jax.experimental.shard_map import shard_map
    from concourse import bass2jax, mybir

    nc = _build_bass()
    bass2jax.install_neuronx_cc_hook()
    partition_name = nc.partition_id_tensor.name if nc.partition_id_tensor else None
    in_names, out_names, out_avals, zero_specs = [], [], [], []
    for alloc in nc.m.functions[0].allocations:
        if not isinstance(alloc, mybir.MemoryLocationSet):
            continue
        name = alloc.memorylocations[0].name
        if alloc.kind == "ExternalInput":
            if name != partition_name:
                in_names.append(name)
        elif alloc.kind == "ExternalOutput":
            shape = tuple(alloc.tensor_shape)
            dtype = mybir.dt.np(alloc.dtype)
            out_names.append(name)
            out_avals.append(jax.core.ShapedArray(shape, dtype))
            zero_specs.append((shape, dtype))
    n_params = len(in_names)
    n_outs = len(out_names)
    all_in_names = tuple(in_names) + tuple(out_names)
    if partition_name is not None:
        all_in_names = all_in_names + (partition_name,)
    donate = tuple(range(n_params, n_params + n_outs))

    def _body(*args):
        operands = list(args)
        if partition_name is not None:
            operands.append(bass2jax.partition_id_tensor())
        outs = bass2jax._bass_exec_p.bind(
            *operands,
            out_avals=tuple(out_avals),
            in_names=all_in_names,
            out_names=tuple(out_names),
            lowering_input_output_aliases=(),
            sim_require_finite=True,
            sim_require_nnan=True,
            nc=nc,
        )
        return tuple(outs)

    devices = jax.devices()[:NCORES]
    assert len(devices) == NCORES
    mesh = Mesh(np.asarray(devices), ("core",))
    in_specs = (PartitionSpec("core"),) * (n_params + n_outs)
    out_specs = (PartitionSpec("core"),) * n_outs
    sharded = jax.jit(
        shard_map(
            _body, mesh=mesh, in_specs=in_specs, out_specs=out_specs,
            check_rep=False,
        ),
        donate_argnums=donate,
        keep_unused=True,
    )
    _CACHE["runner"] = (sharded, in_names, out_names, zero_specs)
    return _CACHE["runner"]


def _tab_template():
    """Input-independent parts of the per-core tab/rows tensors."""
    if "tab_tmpl" in _CACHE:
        return _CACHE["tab_tmpl"]
    p = np.arange(128, dtype=np.float32)
    t64 = np.arange(NCH, dtype=np.float32)
    tab = np.empty((NCORES, 128, TW), np.float32)
    jglob = t64[None, :] * 128.0 + p[:, None]  # [128, 64]
    tab[:, :, 0:NCH] = (
        jglob[None] - (np.arange(NCORES, dtype=np.float32) * M)[:, None, None]
    )
    tab[:, :, NCH : NCH + YC] = (np.arange(YC, dtype=np.float32) - 1.0)[None, None, :]
    tab[:, :, NCH + YC] = (p - 1.0)[None, :]
    rows = np.empty((NCORES, 2, M), np.float32)
    rows[:, 0, :] = np.arange(M, dtype=np.float32)[None, :]
    _CACHE["tab_tmpl"] = (tab, rows)
    return _CACHE["tab_tmpl"]


def _marshal(features: np.ndarray, labels: np.ndarray):
    """Concatenated (axis 0 = core-major) input arrays for shard_map."""
    bf16 = ml_dtypes.bfloat16
    x = np.ascontiguousarray(features, dtype=np.float32)
    nrm = np.sqrt(np.einsum("ij,ij->i", x, x))
    np.maximum(nrm, 1e-12, out=nrm)
    scale = 1.0 / (nrm * np.float32(np.sqrt(TEMPERATURE)))
    xhat = (x * scale[:, None]).astype(bf16)
    # chunk-major x^T: xtc[t, p, dc*128+jj] = xhat[t*128+jj, dc*128+p]
    xtc = np.ascontiguousarray(
        xhat.reshape(NCH, 128, 4, 128).transpose(0, 3, 2, 1)
    ).reshape(NCH, 128, D)

    labf = np.asarray(labels).astype(np.float32)
    tab_tmpl, rows_tmpl = _tab_template()
    tab = tab_tmpl.copy()
    tab[:, :, NCH + YC + 1 : TW] = labf.reshape(NCH, 128).T[None]
    rows = rows_tmpl.copy()
    rows[:, 1, :] = labf.reshape(NCORES, M)
    return {
        "xs": xtc,  # concat of per-core [8, 128, 512] shards == full xtc
        "tab": np.ascontiguousarray(tab.reshape(NCORES * 128, TW)),
        "rows": np.ascontiguousarray(rows.reshape(NCORES * 2, M)),
    }


def kernel(features: np.ndarray, labels: np.ndarray) -> np.ndarray:
    sharded, in_names, out_names, zero_specs = _get_runner()
    arrs = _marshal(features, labels)
    concat_in = [arrs[n] for n in in_names]
    concat_zeros = [
        np.zeros((NCORES * s[0], *s[1:]), dt) for (s, dt) in zero_specs
    ]
    outs = sharded(*concat_in, *concat_zeros)
    loss = np.asarray(outs[0])  # [NCORES * 1, 1]
    return np.float32(loss.sum() / B)


# revision 5
# speedup vs baseline: 15.2401x; 1.1720x over previous
"""Trainium2 Bass kernel for supervised contrastive loss (8-core SPMD).

Math (per reference):
    f = x / max(||x||, 1e-12)            row-normalized features  [B, D]
    s = (f f^T) / TEMP                                            [B, B]
    E = exp(s) with diag zeroed
    P_i = sum_{j != i, l_j == l_i} E_ij   (positives)
    T_i = sum_{j != i} E_ij               (positives + negatives)
    loss = mean_i [ log(T_i + EPS) - log(P_i) ]

Distribution: row-block shard with an on-device AllGather. The host
pre-normalizes (and folds in 1/sqrt(TEMP)) so each core is shipped ONLY its
own 1 MB bf16 shard in chunk-major transposed layout; the full [B, D]
operand is assembled on-device over NeuronLink. This cuts host->device
traffic ~18x vs replicating two full layouts per core, which dominated
end-to-end time under the axon tunnel.

Core c owns rows m in [1024c, 1024(c+1)). For each j-chunk (128 rows) it
computes the E^T block [j x m] with j on the partition dim so both masked
reductions contract over j on the TensorEngine:
    PS1[c', m] = sum_j Y'[j, c'] * E[j, m]     (Y' = one-hot(labels) | ones)
row 0 of PS1 = T_m, and P_m = PS1[l_m+1, m] (recovered with a one-hot
mask + ones-matmul). The diagonal is zeroed with a data-driven mask
(m == t*128 + p - 1024c), so every core runs the identical program with
per-core variation living only in the input data. Per-core scalar partial
losses are summed on host.

The jitted executable is cached in _CACHE: repeat kernel() calls pay only
input marshalling + transfer + device execution.
"""

import numpy as np
import ml_dtypes

TEMPERATURE = 0.07
EPS = 1e-8
B = 8192
D = 512
NCORES = 8
M = B // NCORES          # 1024 rows per core
NCH = B // 128           # 64 j-chunks of 128
BCH = M // 128           # 8 chunks per core shard
NCLS = 100               # label classes
YC = NCLS + 1            # one-hot columns + ones column
TW = NCH + YC + 1 + NCH  # tab cols: jadj | iota_c | iota_p | lab_ch
FP8_SCALE = 64.0         # xhat shipped as fp8e4m3 * FP8_SCALE (elems ~N(0,2.8))
EXP_SCALE = 1.0 / (FP8_SCALE * FP8_SCALE * TEMPERATURE)

_CACHE = {}


def _build_bass():
    import concourse.bacc as bacc
    import concourse.tile as tile
    from concourse import mybir
    from contextlib import ExitStack

    f32 = mybir.dt.float32
    bf16 = mybir.dt.bfloat16
    f8 = mybir.dt.float8e4
    AF = mybir.ActivationFunctionType
    OP = mybir.AluOpType

    nc = bacc.Bacc(num_devices=NCORES)

    # ---- I/O ----------------------------------------------------------
    # xs[t8, p, dc*128+jj] = xhat[(8c+t8)*128+jj, dc*128+p]  (shard, chunk-
    # major x^T; xhat = f / (max(||f||,1e-12) * sqrt(TEMP)) built on host)
    xs_d = nc.declare_dram_parameter("xs", [BCH, 128, D], f8, isOutput=False)
    # tab[:, 0:64]      jadj[p, t] = t*128 + p - c*M
    # tab[:, 64:165]    iota_c[p, i] = i - 1
    # tab[:, 165:166]   iota_p[p] = p - 1
    # tab[:, 166:230]   lab_ch[p, t] = labels[t*128 + p]
    tab_d = nc.declare_dram_parameter("tab", [128, TW], f32, isOutput=False)
    # rows[0, m] = m ; rows[1, m] = labels[c*M + m]
    rows_d = nc.declare_dram_parameter("rows", [2, M], f32, isOutput=False)
    loss_d = nc.declare_dram_parameter("loss", [1, 1], f32, isOutput=True)

    with ExitStack() as ctx:
        tc = ctx.enter_context(tile.TileContext(nc))
        const = ctx.enter_context(tc.tile_pool(name="const", bufs=1))
        ep = ctx.enter_context(tc.tile_pool(name="ep", bufs=3))
        emp = ctx.enter_context(tc.tile_pool(name="emp", bufs=3))
        mkp = ctx.enter_context(tc.tile_pool(name="mkp", bufs=3))
        psum = ctx.enter_context(tc.tile_pool(name="psum", bufs=3, space="PSUM"))
        accp = ctx.enter_context(tc.tile_pool(name="accp", bufs=1, space="PSUM"))
        dram = ctx.enter_context(tc.tile_pool(name="dram", bufs=1, space="DRAM"))

        # ---- all-gather the shard into the full chunk-major x^T -------
        in_b = dram.tile([BCH, 128, D], f8, name="in_b")
        out_b = dram.tile([NCH, 128, D], f8, name="out_b", addr_space="Shared")
        nc.gpsimd.dma_start(out=in_b[:], in_=xs_d[:])
        nc.gpsimd.collective_compute(
            "AllGather",
            OP.bypass,
            replica_groups=[list(range(NCORES))],
            ins=[in_b[:].opt()],
            outs=[out_b[:].opt()],
        )
        # xall[p, t, f] = gathered[t, p, f]: 64 KB/partition, lives in SBUF
        xall = const.tile([128, NCH, D], f8)
        nc.gpsimd.dma_start(out=xall[:], in_=out_b[:].rearrange("t p f -> p t f"))

        # own-block rhs: xnt[p, dc, t8*128+jj] = xs[t8, p, dc*128+jj]
        xnt4 = const.tile([128, 4, BCH, 128], f8)
        nc.sync.dma_start(
            out=xnt4[:], in_=xs_d[:].rearrange("t p (dc j) -> p dc t j", j=128)
        )
        xnt = xnt4[:].rearrange("p dc t j -> p dc (t j)")

        # ---- constants / label machinery ------------------------------
        tab_s = const.tile([128, TW], f32)
        nc.sync.dma_start(out=tab_s[:], in_=tab_d[:])
        jadj = tab_s[:, 0:NCH]
        iota_c = tab_s[:, NCH : NCH + YC]
        iota_p = tab_s[:, NCH + YC : NCH + YC + 1]
        lab_ch = tab_s[:, NCH + YC + 1 : TW]

        # [1, M] rows land on partition 0 of zeroed pads, then are
        # broadcast to all partitions with a ones-matmul (K=128).
        rowpadA = const.tile([128, M], f32)
        nc.vector.memset(rowpadA[:], 0.0)
        nc.sync.dma_start(out=rowpadA[0:1, :], in_=rows_d[0:1, :])
        rowpadB = const.tile([128, M], f32)
        nc.vector.memset(rowpadB[:], 0.0)
        nc.sync.dma_start(out=rowpadB[0:1, :], in_=rows_d[1:2, :])

        ones_f = const.tile([128, 128], f32)
        nc.vector.memset(ones_f[:], 1.0)
        ones101 = const.tile([128, 1], f32)
        nc.vector.memset(ones101[:], 1.0)
        bias_eps = const.tile([128, 1], f32)
        nc.vector.memset(bias_eps[:], EPS)

        miota_ps = psum.tile([128, M], f32, tag="sim", name="miota_ps")
        for h in range(2):
            nc.tensor.matmul(
                miota_ps[:, h * 512 : (h + 1) * 512],
                lhsT=ones_f[:],
                rhs=rowpadA[:, h * 512 : (h + 1) * 512],
                start=True,
                stop=True,
            )
        miota_bc = const.tile([128, M], f32)
        nc.vector.tensor_copy(out=miota_bc[:], in_=miota_ps[:])

        labblk_ps = psum.tile([128, M], f32, tag="sim", name="labblk_ps")
        for h in range(2):
            nc.tensor.matmul(
                labblk_ps[:, h * 512 : (h + 1) * 512],
                lhsT=ones_f[:],
                rhs=rowpadB[:, h * 512 : (h + 1) * 512],
                start=True,
                stop=True,
            )
        labblk_bc = const.tile([128, M], f32)
        nc.vector.tensor_copy(out=labblk_bc[:], in_=labblk_ps[:])

        # Y'[p, t, c'] = (c'-1 == labels[t*128+p]) for c' in 1..100;
        # col 0 = ones (T-sum column, lands on PSUM partition 0).
        yall = const.tile([128, NCH, YC], bf16)
        for t in range(NCH):
            nc.vector.tensor_scalar(
                out=yall[:, t, :], in0=iota_c, scalar1=lab_ch[:, t : t + 1],
                scalar2=None, op0=OP.is_equal,
            )
        nc.vector.memset(yall[:, :, 0:1], 1.0)

        # YblkT[c', m] = (labels[c*M + m] == c'-1)
        yblkt = const.tile([128, M], bf16)
        nc.vector.tensor_scalar(
            out=yblkt[:], in0=labblk_bc[:], scalar1=iota_p,
            scalar2=None, op0=OP.is_equal,
        )

        # ---- main loop over j-chunks ----------------------------------
        ps1 = accp.tile([128, M], f32)  # row 0: T; rows 1..100: class sums
        for t in range(NCH):
            ps = psum.tile([128, M], f32, tag="sim")
            for dc in range(4):
                for h in range(2):
                    nc.tensor.matmul(
                        ps[:, h * 512 : (h + 1) * 512],
                        lhsT=xall[:, t, dc * 128 : (dc + 1) * 128],
                        rhs=xnt[:, dc, h * 512 : (h + 1) * 512],
                        start=(dc == 0),
                        stop=(dc == 3),
                    )
            e_t = ep.tile([128, M], bf16)
            nc.scalar.activation(out=e_t[:], in_=ps[:], func=AF.Exp, scale=EXP_SCALE)
            # diag mask: zero (p, m) where m == t*128 + p - c*M
            mask_t = mkp.tile([128, M], bf16)
            nc.vector.tensor_scalar(
                out=mask_t[:], in0=miota_bc[:], scalar1=jadj[:, t : t + 1],
                scalar2=None, op0=OP.not_equal,
            )
            em_t = emp.tile([128, M], bf16)
            nc.vector.tensor_mul(out=em_t[:], in0=e_t[:], in1=mask_t[:])
            for h in range(2):
                nc.tensor.matmul(
                    ps1[0:YC, h * 512 : (h + 1) * 512],
                    lhsT=yall[:, t, :],
                    rhs=em_t[:, h * 512 : (h + 1) * 512],
                    start=(t == 0),
                    stop=(t == NCH - 1),
                )

        # ---- finalize: P via one-hot mask + partition reduce ----------
        maskd = const.tile([128, M], f32)
        nc.vector.tensor_tensor(
            out=maskd[0:YC, :], in0=ps1[0:YC, :], in1=yblkt[0:YC, :], op=OP.mult
        )
        pps = psum.tile([128, M], f32, tag="sim", name="pps")
        for h in range(2):
            nc.tensor.matmul(
                pps[0:1, h * 512 : (h + 1) * 512],
                lhsT=ones101[0:YC, 0:1],
                rhs=maskd[0:YC, h * 512 : (h + 1) * 512],
                start=True,
                stop=True,
            )
        ln_t = const.tile([1, M], f32)
        nc.scalar.activation(
            out=ln_t[:], in_=ps1[0:1, :], func=AF.Ln, bias=bias_eps[0:1, :]
        )
        ln_p = const.tile([1, M], f32)
        nc.scalar.activation(out=ln_p[:], in_=pps[0:1, :], func=AF.Ln)
        diff = const.tile([1, M], f32)
        nc.vector.tensor_sub(out=diff[:], in0=ln_t[:], in1=ln_p[:])
        losss = const.tile([1, 1], f32)
        nc.vector.tensor_reduce(
            out=losss[:], in_=diff[:], axis=mybir.AxisListType.X, op=OP.add
        )
        nc.sync.dma_start(out=loss_d[:], in_=losss[:])

    # Bacc.finalize() runs the wait-splitting / ldweights / act-table /
    # extended-ISA codegen passes that walrus requires.
    nc.finalize()
    return nc


def _get_runner():
    """Build the Bass program and a CACHED jitted SPMD executable.

    run_bass_kernel_spmd builds a fresh jit closure per call (full retrace +
    XLA compile every time); caching the executable makes repeat kernel()
    calls pay only marshal + transfer + execute.
    """
    if "runner" in _CACHE:
        return _CACHE["runner"]
    import jax
    from jax.sharding import Mesh, PartitionSpec
    from jax.experimental.shard_map import shard_map
    from concourse import bass2jax, mybir

    nc = _build_bass()
    bass2jax.install_neuronx_cc_hook()
    partition_name = nc.partition_id_tensor.name if nc.partition_id_tensor else None
    in_names, out_names, out_avals, zero_specs = [], [], [], []
    for alloc in nc.m.functions[0].allocations:
        if not isinstance(alloc, mybir.MemoryLocationSet):
            continue
        name = alloc.memorylocations[0].name
        if alloc.kind == "ExternalInput":
            if name != partition_name:
                in_names.append(name)
        elif alloc.kind == "ExternalOutput":
            shape = tuple(alloc.tensor_shape)
            dtype = mybir.dt.np(alloc.dtype)
            out_names.append(name)
            out_avals.append(jax.core.ShapedArray(shape, dtype))
            zero_specs.append((shape, dtype))
    n_params = len(in_names)
    n_outs = len(out_names)
    all_in_names = tuple(in_names) + tuple(out_names)
    if partition_name is not None:
        all_in_names = all_in_names + (partition_name,)
    donate = tuple(range(n_params, n_params + n_outs))

    def _body(*args):
        operands = list(args)
        if partition_name is not None:
            operands.append(bass2jax.partition_id_tensor())
        outs = bass2jax._bass_exec_p.bind(
            *operands,
            out_avals=tuple(out_avals),
            in_names=all_in_names,
            out_names=tuple(out_names),
            lowering_input_output_aliases=(),
            sim_require_finite=True,
            sim_require_nnan=True,
            nc=nc,
        )
        return tuple(outs)

    devices = jax.devices()[:NCORES]
    assert len(devices) == NCORES
    mesh = Mesh(np.asarray(devices), ("core",))
    in_specs = (PartitionSpec("core"),) * (n_params + n_outs)
    out_specs = (PartitionSpec("core"),) * n_outs
    sharded = jax.jit(
        shard_map(
            _body, mesh=mesh, in_specs=in_specs, out_specs=out_specs,
            check_rep=False,
        ),
        donate_argnums=donate,
        keep_unused=True,
    )
    _CACHE["runner"] = (sharded, in_names, out_names, zero_specs)
    return _CACHE["runner"]


def _tab_template():
    """Input-independent parts of the per-core tab/rows tensors."""
    if "tab_tmpl" in _CACHE:
        return _CACHE["tab_tmpl"]
    p = np.arange(128, dtype=np.float32)
    t64 = np.arange(NCH, dtype=np.float32)
    tab = np.empty((NCORES, 128, TW), np.float32)
    jglob = t64[None, :] * 128.0 + p[:, None]  # [128, 64]
    tab[:, :, 0:NCH] = (
        jglob[None] - (np.arange(NCORES, dtype=np.float32) * M)[:, None, None]
    )
    tab[:, :, NCH : NCH + YC] = (np.arange(YC, dtype=np.float32) - 1.0)[None, None, :]
    tab[:, :, NCH + YC] = (p - 1.0)[None, :]
    rows = np.empty((NCORES, 2, M), np.float32)
    rows[:, 0, :] = np.arange(M, dtype=np.float32)[None, :]
    _CACHE["tab_tmpl"] = (tab, rows)
    return _CACHE["tab_tmpl"]


def _marshal(features: np.ndarray, labels: np.ndarray):
    """Concatenated (axis 0 = core-major) input arrays for shard_map."""
    f8 = ml_dtypes.float8_e4m3
    x = np.ascontiguousarray(features, dtype=np.float32)
    nrm = np.sqrt(np.einsum("ij,ij->i", x, x))
    np.maximum(nrm, 1e-12, out=nrm)
    scale = np.float32(FP8_SCALE) / nrm
    xhat = (x * scale[:, None]).astype(f8)
    # chunk-major x^T: xtc[t, p, dc*128+jj] = xhat[t*128+jj, dc*128+p]
    xtc = np.ascontiguousarray(
        xhat.reshape(NCH, 128, 4, 128).transpose(0, 3, 2, 1)
    ).reshape(NCH, 128, D)

    labf = np.asarray(labels).astype(np.float32)
    tab_tmpl, rows_tmpl = _tab_template()
    tab = tab_tmpl.copy()
    tab[:, :, NCH + YC + 1 : TW] = labf.reshape(NCH, 128).T[None]
    rows = rows_tmpl.copy()
    rows[:, 1, :] = labf.reshape(NCORES, M)
    return {
        "xs": xtc,  # concat of per-core [8, 128, 512] shards == full xtc
        "tab": np.ascontiguousarray(tab.reshape(NCORES * 128, TW)),
        "rows": np.ascontiguousarray(rows.reshape(NCORES * 2, M)),
    }


def kernel(features: np.ndarray, labels: np.ndarray) -> np.ndarray:
    sharded, in_names, out_names, zero_specs = _get_runner()
    arrs = _marshal(features, labels)
    concat_in = [arrs[n] for n in in_names]
    concat_zeros = [
        np.zeros((NCORES * s[0], *s[1:]), dt) for (s, dt) in zero_specs
    ]
    outs = sharded(*concat_in, *concat_zeros)
    loss = np.asarray(outs[0])  # [NCORES * 1, 1]
    return np.float32(loss.sum() / B)


# revision 7
# speedup vs baseline: 19.7331x; 1.2948x over previous
"""Trainium2 Bass kernel for supervised contrastive loss (8-core SPMD).

Math (per reference):
    f = x / max(||x||, 1e-12)            row-normalized features  [B, D]
    s = (f f^T) / TEMP                                            [B, B]
    E = exp(s) with diag zeroed
    P_i = sum_{j != i, l_j == l_i} E_ij   (positives)
    T_i = sum_{j != i} E_ij               (positives + negatives)
    loss = mean_i [ log(T_i + EPS) - log(P_i) ]

Distribution: row-block shard with an on-device AllGather. The host
pre-normalizes and quantizes to fp8e4m3 (x FP8_SCALE; the descale is folded
into the exp), so each core is shipped ONLY its own 512 KB shard in
chunk-major transposed layout; the full [B, D] operand is assembled
on-device over NeuronLink. End-to-end time under the axon tunnel is
transfer-dominated, so payload bytes are the currency: ~4.4 MB total vs
163 MB for the replicate-everything baseline.

Core c owns rows m in [1024c, 1024(c+1)). For each j-chunk (128 rows) it
computes the E^T block [j x m] with j on the partition dim so both masked
reductions contract over j on the TensorEngine:
    PS1[c', m] = sum_j Y'[j, c'] * E[j, m]     (Y' = one-hot(labels) | ones)
row 0 of PS1 = T_m, and P_m = PS1[l_m+1, m] (recovered with a one-hot
mask + ones-matmul). The diagonal is zeroed with a data-driven mask
(m == t*128 + p - 1024c), so every core runs the identical program with
per-core variation living only in the input data. Per-core partial losses
are AllReduce-summed on device; the host fetches one replicated scalar.

The jitted executable is cached in _CACHE: repeat kernel() calls pay only
input marshalling + transfer + device execution.
"""

import numpy as np
import ml_dtypes

TEMPERATURE = 0.07
EPS = 1e-8
B = 8192
D = 512
NCORES = 8
M = B // NCORES          # 1024 rows per core
NCH = B // 128           # 64 j-chunks of 128
BCH = M // 128           # 8 chunks per core shard
NCLS = 100               # label classes
YC = NCLS + 1            # one-hot columns + ones column
FP8_SCALE = 64.0         # xhat shipped as fp8e4m3 * FP8_SCALE (elems ~N(0,2.8))
EXP_SCALE = 1.0 / (FP8_SCALE * FP8_SCALE * TEMPERATURE)

_CACHE = {}


def _build_bass():
    import concourse.bacc as bacc
    import concourse.tile as tile
    from concourse import mybir
    from contextlib import ExitStack

    f32 = mybir.dt.float32
    bf16 = mybir.dt.bfloat16
    f8 = mybir.dt.float8e4
    AF = mybir.ActivationFunctionType
    OP = mybir.AluOpType

    nc = bacc.Bacc(num_devices=NCORES)

    # ---- I/O ----------------------------------------------------------
    # xs[t8, p, dc*128+jj] = xhat[(8c+t8)*128+jj, dc*128+p]  (fp8 shard,
    # chunk-major x^T; xhat = FP8_SCALE * f / max(||f||,1e-12))
    xs_d = nc.declare_dram_parameter("xs", [BCH, 128, D], f8, isOutput=False)
    # labt[p, t] = labels[t*128 + p]  (bf16; labels < 256 are exact)
    labt_d = nc.declare_dram_parameter("labt", [128, NCH], bf16, isOutput=False)
    # tab[:, 0] = p - 1 ; tab[:, 1] = p
    tab_d = nc.declare_dram_parameter("tab", [128, 2], f32, isOutput=False)
    # rows[0, m] = m ; rows[1, m] = labels[c*M + m] ;
    # rows[2, 0:64] = t*128 - c*M ; rows[2, 64:165] = i - 1 (iota_c)
    rows_d = nc.declare_dram_parameter("rows", [3, M], f32, isOutput=False)
    loss_d = nc.declare_dram_parameter("loss", [1, 1], f32, isOutput=True)

    with ExitStack() as ctx:
        tc = ctx.enter_context(tile.TileContext(nc))
        const = ctx.enter_context(tc.tile_pool(name="const", bufs=1))
        ep = ctx.enter_context(tc.tile_pool(name="ep", bufs=3))
        emp = ctx.enter_context(tc.tile_pool(name="emp", bufs=3))
        mkp = ctx.enter_context(tc.tile_pool(name="mkp", bufs=3))
        psum = ctx.enter_context(tc.tile_pool(name="psum", bufs=3, space="PSUM"))
        accp = ctx.enter_context(tc.tile_pool(name="accp", bufs=1, space="PSUM"))
        dram = ctx.enter_context(tc.tile_pool(name="dram", bufs=1, space="DRAM"))

        # ---- all-gather the shard into the full chunk-major x^T -------
        in_b = dram.tile([BCH, 128, D], f8, name="in_b")
        out_b = dram.tile([NCH, 128, D], f8, name="out_b", addr_space="Shared")
        nc.gpsimd.dma_start(out=in_b[:], in_=xs_d[:])
        nc.gpsimd.collective_compute(
            "AllGather",
            OP.bypass,
            replica_groups=[list(range(NCORES))],
            ins=[in_b[:].opt()],
            outs=[out_b[:].opt()],
        )
        # xall[p, t, f] = gathered[t, p, f]: 32 KB/partition, lives in SBUF
        xall = const.tile([128, NCH, D], f8)
        nc.gpsimd.dma_start(out=xall[:], in_=out_b[:].rearrange("t p f -> p t f"))

        # own-block rhs: xnt[p, dc, t8*128+jj] = xs[t8, p, dc*128+jj]
        xnt4 = const.tile([128, 4, BCH, 128], f8)
        nc.sync.dma_start(
            out=xnt4[:], in_=xs_d[:].rearrange("t p (dc j) -> p dc t j", j=128)
        )
        xnt = xnt4[:].rearrange("p dc t j -> p dc (t j)")

        # ---- constants / label machinery ------------------------------
        labt_s = const.tile([128, NCH], bf16)
        nc.sync.dma_start(out=labt_s[:], in_=labt_d[:])
        tab_s = const.tile([128, 2], f32)
        nc.sync.dma_start(out=tab_s[:], in_=tab_d[:])
        iota_p = tab_s[:, 0:1]
        iota_p0 = tab_s[:, 1:2]

        # [1, M] rows land on partition 0 of zeroed pads, then are
        # broadcast to all partitions with a ones-matmul (K=128).
        ones_f = const.tile([128, 128], f32)
        nc.vector.memset(ones_f[:], 1.0)
        ones101 = const.tile([128, 1], f32)
        nc.vector.memset(ones101[:], 1.0)
        bias_eps = const.tile([128, 1], f32)
        nc.vector.memset(bias_eps[:], EPS)

        bcs = []  # miota_bc, labblk_bc, misc_bc
        for r in range(3):
            rowpad = const.tile([128, M], f32, name=f"rowpad{r}")
            nc.vector.memset(rowpad[:], 0.0)
            nc.sync.dma_start(out=rowpad[0:1, :], in_=rows_d[r : r + 1, :])
            bc_ps = psum.tile([128, M], f32, tag="sim", name=f"bc_ps{r}")
            for h in range(2):
                nc.tensor.matmul(
                    bc_ps[:, h * 512 : (h + 1) * 512],
                    lhsT=ones_f[:],
                    rhs=rowpad[:, h * 512 : (h + 1) * 512],
                    start=True,
                    stop=True,
                )
            bc = const.tile([128, M], f32, name=f"bc{r}")
            nc.vector.tensor_copy(out=bc[:], in_=bc_ps[:])
            bcs.append(bc)
        miota_bc, labblk_bc, misc_bc = bcs

        # jadj[p, t] = t*128 + p - c*M
        jadj = const.tile([128, NCH], f32)
        nc.vector.tensor_scalar(
            out=jadj[:], in0=misc_bc[:, 0:NCH], scalar1=iota_p0,
            scalar2=None, op0=OP.add,
        )
        # labels as f32 (tensor_scalar requires an f32 scalar1)
        labt_f = const.tile([128, NCH], f32)
        nc.vector.tensor_copy(out=labt_f[:], in_=labt_s[:])

        # Y'[p, t, c'] = (c'-1 == labels[t*128+p]) for c' in 1..100;
        # col 0 = ones (T-sum column, lands on PSUM partition 0).
        yall = const.tile([128, NCH, YC], bf16)
        for t in range(NCH):
            nc.vector.tensor_scalar(
                out=yall[:, t, :], in0=misc_bc[:, NCH : NCH + YC],
                scalar1=labt_f[:, t : t + 1],
                scalar2=None, op0=OP.is_equal,
            )
        nc.vector.memset(yall[:, :, 0:1], 1.0)

        # YblkT[c', m] = (labels[c*M + m] == c'-1)
        yblkt = const.tile([128, M], bf16)
        nc.vector.tensor_scalar(
            out=yblkt[:], in0=labblk_bc[:], scalar1=iota_p,
            scalar2=None, op0=OP.is_equal,
        )

        # ---- main loop over j-chunks ----------------------------------
        ps1 = accp.tile([128, M], f32)  # row 0: T; rows 1..100: class sums
        for t in range(NCH):
            ps = psum.tile([128, M], f32, tag="sim")
            for dc in range(4):
                for h in range(2):
                    nc.tensor.matmul(
                        ps[:, h * 512 : (h + 1) * 512],
                        lhsT=xall[:, t, dc * 128 : (dc + 1) * 128],
                        rhs=xnt[:, dc, h * 512 : (h + 1) * 512],
                        start=(dc == 0),
                        stop=(dc == 3),
                    )
            e_t = ep.tile([128, M], bf16)
            nc.scalar.activation(out=e_t[:], in_=ps[:], func=AF.Exp, scale=EXP_SCALE)
            # diag mask: zero (p, m) where m == t*128 + p - c*M
            mask_t = mkp.tile([128, M], bf16)
            nc.vector.tensor_scalar(
                out=mask_t[:], in0=miota_bc[:], scalar1=jadj[:, t : t + 1],
                scalar2=None, op0=OP.not_equal,
            )
            em_t = emp.tile([128, M], bf16)
            nc.vector.tensor_mul(out=em_t[:], in0=e_t[:], in1=mask_t[:])
            for h in range(2):
                nc.tensor.matmul(
                    ps1[0:YC, h * 512 : (h + 1) * 512],
                    lhsT=yall[:, t, :],
                    rhs=em_t[:, h * 512 : (h + 1) * 512],
                    start=(t == 0),
                    stop=(t == NCH - 1),
                )

        # ---- finalize: P via one-hot mask + partition reduce ----------
        maskd = const.tile([128, M], f32)
        nc.vector.tensor_tensor(
            out=maskd[0:YC, :], in0=ps1[0:YC, :], in1=yblkt[0:YC, :], op=OP.mult
        )
        pps = psum.tile([128, M], f32, tag="sim", name="pps")
        for h in range(2):
            nc.tensor.matmul(
                pps[0:1, h * 512 : (h + 1) * 512],
                lhsT=ones101[0:YC, 0:1],
                rhs=maskd[0:YC, h * 512 : (h + 1) * 512],
                start=True,
                stop=True,
            )
        ln_t = const.tile([1, M], f32)
        nc.scalar.activation(
            out=ln_t[:], in_=ps1[0:1, :], func=AF.Ln, bias=bias_eps[0:1, :]
        )
        ln_p = const.tile([1, M], f32)
        nc.scalar.activation(out=ln_p[:], in_=pps[0:1, :], func=AF.Ln)
        diff = const.tile([1, M], f32)
        nc.vector.tensor_sub(out=diff[:], in0=ln_t[:], in1=ln_p[:])
        lr_pad = const.tile([1, 128], f32)
        nc.vector.memset(lr_pad[:], 0.0)
        nc.vector.tensor_reduce(
            out=lr_pad[0:1, 0:1], in_=diff[:], axis=mybir.AxisListType.X, op=OP.add
        )
        # AllReduce the per-core partial so every core holds the total and
        # the host fetches one replicated scalar (no 8-shard gather).
        lr_in = dram.tile([1, 128], f32, name="lr_in")
        lr_out = dram.tile([1, 128], f32, name="lr_out", addr_space="Shared")
        nc.gpsimd.dma_start(out=lr_in[:], in_=lr_pad[:])
        nc.gpsimd.collective_compute(
            "AllReduce",
            OP.add,
            replica_groups=[list(range(NCORES))],
            ins=[lr_in[:].opt()],
            outs=[lr_out[:].opt()],
        )
        nc.gpsimd.dma_start(out=loss_d[:], in_=lr_out[0:1, 0:1])

    # Bacc.finalize() runs the wait-splitting / ldweights / act-table /
    # extended-ISA codegen passes that walrus requires.
    nc.finalize()
    return nc


def _get_runner():
    """Build the Bass program and a CACHED jitted SPMD executable.

    run_bass_kernel_spmd builds a fresh jit closure per call (full retrace +
    XLA compile every time); caching the executable makes repeat kernel()
    calls pay only marshal + transfer + execute.
    """
    if "runner" in _CACHE:
        return _CACHE["runner"]
    import jax
    from jax.sharding import Mesh, PartitionSpec
    from jax.experimental.shard_map import shard_map
    from concourse import bass2jax, mybir

    nc = _build_bass()
    bass2jax.install_neuronx_cc_hook()
    partition_name = nc.partition_id_tensor.name if nc.partition_id_tensor else None
    in_names, out_names, out_avals, zero_specs = [], [], [], []
    for alloc in nc.m.functions[0].allocations:
        if not isinstance(alloc, mybir.MemoryLocationSet):
            continue
        name = alloc.memorylocations[0].name
        if alloc.kind == "ExternalInput":
            if name != partition_name:
                in_names.append(name)
        elif alloc.kind == "ExternalOutput":
            shape = tuple(alloc.tensor_shape)
            dtype = mybir.dt.np(alloc.dtype)
            out_names.append(name)
            out_avals.append(jax.core.ShapedArray(shape, dtype))
            zero_specs.append((shape, dtype))
    n_params = len(in_names)
    n_outs = len(out_names)
    all_in_names = tuple(in_names) + tuple(out_names)
    if partition_name is not None:
        all_in_names = all_in_names + (partition_name,)
    donate = tuple(range(n_params, n_params + n_outs))

    def _body(*args):
        operands = list(args)
        if partition_name is not None:
            operands.append(bass2jax.partition_id_tensor())
        outs = bass2jax._bass_exec_p.bind(
            *operands,
            out_avals=tuple(out_avals),
            in_names=all_in_names,
            out_names=tuple(out_names),
            lowering_input_output_aliases=(),
            sim_require_finite=True,
            sim_require_nnan=True,
            nc=nc,
        )
        return tuple(outs)

    devices = jax.devices()[:NCORES]
    assert len(devices) == NCORES
    mesh = Mesh(np.asarray(devices), ("core",))
    in_specs = (PartitionSpec("core"),) * (n_params + n_outs)
    # outputs are AllReduce-replicated on device; fetch one copy
    out_specs = (PartitionSpec(),) * n_outs
    sharded = jax.jit(
        shard_map(
            _body, mesh=mesh, in_specs=in_specs, out_specs=out_specs,
            check_rep=False,
        ),
        donate_argnums=donate,
        keep_unused=True,
    )
    _CACHE["runner"] = (sharded, in_names, out_names, zero_specs)
    return _CACHE["runner"]


def _static_tabs():
    """Input-independent marshalling pieces (cached)."""
    if "static" in _CACHE:
        return _CACHE["static"]
    # f16 -> fp8e4m3 byte LUT (indexed by the raw uint16 bits)
    lut = (
        np.arange(65536, dtype=np.uint16)
        .view(np.float16)
        .astype(np.float32)
        .astype(ml_dtypes.float8_e4m3)
        .view(np.uint8)
    )
    p = np.arange(128, dtype=np.float32)
    tab = np.empty((NCORES, 128, 2), np.float32)
    tab[:, :, 0] = (p - 1.0)[None, :]
    tab[:, :, 1] = p[None, :]
    rows = np.zeros((NCORES, 3, M), np.float32)
    rows[:, 0, :] = np.arange(M, dtype=np.float32)[None, :]
    t64 = np.arange(NCH, dtype=np.float32) * 128.0
    rows[:, 2, 0:NCH] = t64[None, :] - (
        np.arange(NCORES, dtype=np.float32) * M
    )[:, None]
    rows[:, 2, NCH : NCH + YC] = (np.arange(YC, dtype=np.float32) - 1.0)[None, :]
    _CACHE["static"] = (lut, tab.reshape(NCORES * 128, 2), rows)
    return _CACHE["static"]


def _marshal(features: np.ndarray, labels: np.ndarray):
    """Concatenated (axis 0 = core-major) input arrays for shard_map."""
    lut, tab_flat, rows_tmpl = _static_tabs()
    x = np.ascontiguousarray(features, dtype=np.float32)
    nrm = np.sqrt(np.einsum("ij,ij->i", x, x))
    np.maximum(nrm, 1e-12, out=nrm)
    scale = np.float32(FP8_SCALE) / nrm
    # f32 -> f16 (SIMD) -> fp8 bytes via LUT, transpose fused into the gather
    x16 = (x * scale[:, None]).astype(np.float16)
    q = lut[x16.view(np.uint16)]  # uint8 [B, D]
    xtc = np.ascontiguousarray(
        q.reshape(NCH, 128, 4, 128).transpose(0, 3, 2, 1)
    ).reshape(NCH, 128, D).view(ml_dtypes.float8_e4m3)

    labf = np.asarray(labels).astype(np.float32)
    labt = np.broadcast_to(
        labf.reshape(NCH, 128).T.astype(ml_dtypes.bfloat16)[None], (NCORES, 128, NCH)
    )
    rows = rows_tmpl.copy()
    rows[:, 1, :] = labf.reshape(NCORES, M)
    return {
        "xs": xtc,  # concat of per-core [8, 128, 512] shards == full xtc
        "labt": np.ascontiguousarray(labt).reshape(NCORES * 128, NCH),
        "tab": tab_flat,
        "rows": rows.reshape(NCORES * 3, M),
    }


def kernel(features: np.ndarray, labels: np.ndarray) -> np.ndarray:
    sharded, in_names, out_names, zero_specs = _get_runner()
    arrs = _marshal(features, labels)
    concat_in = [arrs[n] for n in in_names]
    concat_zeros = [
        np.zeros((NCORES * s[0], *s[1:]), dt) for (s, dt) in zero_specs
    ]
    outs = sharded(*concat_in, *concat_zeros)
    total = float(np.asarray(outs[0]).reshape(-1)[0])
    return np.float32(total / B)
